# revision 1
# baseline (speedup 1.0000x reference)
"""BinarySEResBlock on 8 trn2 NeuronCores.

Reference computation:
  out = hardtanh(BN1(conv1d(x, sign(w1))))            # training-mode BN over (B, L)
  out = SE(BN2(conv1d(out, sign(w2))))                # SE: sigmoid-MLP channel scale
  out = hardtanh(out + x)

Strategy: data-parallel over batch (32 samples -> 4 per core).
 - Convs run as bf16 matmuls on the tensor engine (sign(w) is exact in bf16;
   3 taps = 3 shifted matmuls accumulated in PSUM, contraction = 2 x 128
   channel blocks).  rel_l2 vs fp32 reference ~1.5e-3 (validated offline).
 - BN batch stats are global: per-channel sum/sumsq via bn_stats/bn_aggr,
   then a 2KB AllReduce (twice).
 - SE block is per-sample -> fully local. MLP runs in fp32 on the PE.
 - conv2's raw output spills to HBM (bf16) under conv2's own compute window,
   then reloads into the freed conv2-input slot during the second AllReduce;
   x stays SBUF-resident (bf16) for the residual.
 - Final pass: hardtanh(alpha*conv2 + beta + x) as ACT affine + DVE add/clip,
   bf16 output DMA (host upcasts to f32).
 - Junk matmuls pinned behind each conv hold the PE's HAM clock gate open
   across the AllReduce windows.

Layouts (per core):
  x        [4, 256, 4096] f32   (batch shard)
  w1t/w2t  [128, 3, 2, 2, 128] bf16 : [ci, k, p(cin blk), q(cout blk), co]
  gb1/gb2  [128, 2, 2] f32 : [ci, q, {gamma, beta}]
  fc1t     [128, 2, 64] f32 : lhsT for s @ fc1.T  (contraction C=256)
  fc2t     [64, 2, 128] f32 : lhsT for s1 @ fc2.T (contraction 64)
  out      [4, 256, 4096] f32
"""
import sys
sys.path.insert(0, '/opt/trn_rl_repo')

import numpy as np
import ml_dtypes

import concourse.bass as bass
from concourse import bacc
import concourse.tile as tile
from concourse import mybir
from concourse.bass_utils import run_bass_kernel_spmd

F32 = mybir.dt.float32
BF16 = mybir.dt.bfloat16
OP = mybir.AluOpType
AF = mybir.ActivationFunctionType

NCORES = 8
B = 4              # samples per core
C = 256            # channels
CB = 2             # channel blocks of 128
L = 4096
PADL = L + 2       # one zero column each side per sample
T = 8              # 512-wide l-tiles per sample
TN = 512
K = 3
NLOC = B * L                      # per-core elements per channel
NGLOB = NCORES * NLOC             # global elements per channel
EPS = 1e-5

PHASES = ["conv1", "ar1", "conv2", "ar2", "se", "full"]


def _emit_rsqrt(nc, sb, veps, out_ap):
    """out = 1/sqrt(veps).  ACT sqrt (loose ULP) + DVE reciprocal, then one
    Newton step on rsqrt: r1 = r0*(1.5 - 0.5*v*r0^2)."""
    s0 = sb.tile(list(veps.shape), F32, tag="rs_s0", name="rs_s0")
    nc.scalar.activation(out=s0, in_=veps, func=AF.Sqrt, bias=0.0, scale=1.0)
    r0 = sb.tile(list(veps.shape), F32, tag="rs_r0", name="rs_r0")
    nc.vector.reciprocal(out=r0, in_=s0)
    t = sb.tile(list(veps.shape), F32, tag="rs_t", name="rs_t")
    nc.vector.tensor_tensor(out=t, in0=r0, in1=r0, op=OP.mult)
    nc.vector.tensor_tensor(out=t, in0=t, in1=veps, op=OP.mult)
    nc.vector.tensor_scalar(out=t, in0=t, scalar1=-0.5, scalar2=1.5,
                            op0=OP.mult, op1=OP.add)
    nc.vector.tensor_tensor(out=out_ap, in0=t, in1=r0, op=OP.mult)


def _emit_bn_params(nc, sb, red, gb, ab_out):
    """red [128, CB, 2] = all-reduced {sum, sumsq}; gb [128, CB, 2] = {gamma,
    beta}.  ab_out [128, CB, 2] <- {a = gamma*rsqrt(var+eps), b = beta - mean*a}."""
    inv_n = 1.0 / float(NGLOB)
    mg = sb.tile([128, CB], F32, tag="bn_mg", name="bn_mg")
    nc.vector.tensor_scalar_mul(out=mg, in0=red[:, :, 0], scalar1=inv_n)
    e2 = sb.tile([128, CB], F32, tag="bn_e2", name="bn_e2")
    nc.vector.tensor_scalar_mul(out=e2, in0=red[:, :, 1], scalar1=inv_n)
    var = sb.tile([128, CB], F32, tag="bn_var", name="bn_var")
    nc.vector.tensor_tensor(out=var, in0=mg, in1=mg, op=OP.mult)
    nc.vector.tensor_tensor(out=var, in0=e2, in1=var, op=OP.subtract)
    nc.vector.tensor_scalar_add(out=var, in0=var, scalar1=EPS)
    rst = sb.tile([128, CB], F32, tag="bn_rst", name="bn_rst")
    _emit_rsqrt(nc, sb, var, rst)
    nc.vector.tensor_tensor(out=ab_out[:, :, 0], in0=gb[:, :, 0], in1=rst, op=OP.mult)
    t = sb.tile([128, CB], F32, tag="bn_t", name="bn_t")
    nc.vector.tensor_tensor(out=t, in0=mg, in1=ab_out[:, :, 0], op=OP.mult)
    nc.vector.tensor_tensor(out=ab_out[:, :, 1], in0=gb[:, :, 1], in1=t, op=OP.subtract)


def _emit_stats_payload(nc, sb, stats, pay):
    """stats [128, CB, B*T, 6] bn_stats entries -> pay [128, CB, 2] local
    {sum, sumsq}.  All tiles are 512 elements so bn_aggr's equal-count
    combine is exact."""
    n = float(NLOC)
    for q in range(CB):
        agg = sb.tile([128, 2], F32, tag="pay_agg", name="pay_agg")
        nc.vector.bn_aggr(out=agg, in_=stats[:, q, :, :])
        nc.vector.tensor_scalar_mul(out=pay[:, q, 0:1], in0=agg[:, 0:1], scalar1=n)
        t = sb.tile([128, 1], F32, tag="pay_t", name="pay_t")
        nc.vector.tensor_tensor(out=t, in0=agg[:, 0:1], in1=agg[:, 0:1], op=OP.mult)
        nc.vector.tensor_tensor(out=t, in0=agg[:, 1:2], in1=t, op=OP.add)
        nc.vector.tensor_scalar_mul(out=pay[:, q, 1:2], in0=t, scalar1=n)


def _emit_conv(nc, ps, w_sb, rhs, b, stats, halfgroup=8,
               dst=None, dst_off=0, spill=None, spill_pool=None):
    """One sample's conv for both cout blocks.
    rhs:  [128, CB, B, PADL] bf16 (padded input, partition = cin)
    Either dst (SBUF bf16 tile, writes [:, q, b, dst_off + t*TN:...]) or
    spill (DRAM [CB, B, 128, L] bf16 + spill_pool for the bounce ring).
    stats: [128, CB, B*T, 6] bn_stats accumulator."""
    for q in range(CB):
        for h in range(0, T, halfgroup):
            pts = []
            for t in range(h, h + halfgroup):
                pt = ps.tile([128, TN], F32, tag="pt", name="conv_pt")
                pts.append(pt)
            for p in range(CB):
                for k in range(K):
                    first = (p == 0 and k == 0)
                    last = (p == CB - 1 and k == K - 1)
                    for i, t in enumerate(range(h, h + halfgroup)):
                        nc.tensor.matmul(
                            pts[i],
                            w_sb[:, k, p, q, :],
                            rhs[:, p, b, t * TN + k: t * TN + k + TN],
                            start=first, stop=last)
            if spill is not None:
                sp = spill_pool.tile([128, halfgroup * TN], BF16, tag="c2sp",
                                     name="c2sp", bufs=2)
                for i, t in enumerate(range(h, h + halfgroup)):
                    nc.scalar.copy(sp[:, i * TN:(i + 1) * TN], pts[i])
                    nc.vector.bn_stats(out=stats[:, q, b * T + t, :], in_=pts[i])
                nc.sync.dma_start(
                    out=spill[q, b, :, h * TN: (h + halfgroup) * TN], in_=sp)
            else:
                for i, t in enumerate(range(h, h + halfgroup)):
                    nc.scalar.copy(
                        dst[:, q, b, dst_off + t * TN: dst_off + (t + 1) * TN],
                        pts[i])
                    nc.vector.bn_stats(out=stats[:, q, b * T + t, :], in_=pts[i])


def _emit_warm(nc, ps, lhsT, rhs, n):
    """Junk matmuls to hold the PE's HAM clock gate open across an
    AllReduce window (PE is FIFO: these run right after the preceding
    conv's last matmul)."""
    import math as _m
    nfree = 1
    for _, cnt in rhs.ap:
        nfree *= cnt
    nfree = nfree // 128 if rhs.shape[0] == 128 else nfree
    nfree = min(nfree, 512)
    for _ in range(n):
        warm = ps.tile([128, 512], F32, tag="pt", name="conv_pt")
        nc.tensor.matmul(warm[:, :nfree], lhsT, rhs, start=True, stop=True)


def build(stop_after="full"):
    lvl = PHASES.index(stop_after)
    nc = bacc.Bacc(num_devices=NCORES)

    x_d = nc.declare_dram_parameter("x", [B, C, L], F32, isOutput=False)
    w1_d = nc.declare_dram_parameter("w1t", [128, K, CB, CB, 128], BF16, isOutput=False)
    w2_d = nc.declare_dram_parameter("w2t", [128, K, CB, CB, 128], BF16, isOutput=False)
    gb1_d = nc.declare_dram_parameter("gb1", [128, CB, 2], F32, isOutput=False)
    gb2_d = nc.declare_dram_parameter("gb2", [128, CB, 2], F32, isOutput=False)
    fc1_d = nc.declare_dram_parameter("fc1t", [128, CB, 64], F32, isOutput=False)
    fc2_d = nc.declare_dram_parameter("fc2t", [64, CB, 128], F32, isOutput=False)
    out_d = nc.declare_dram_parameter("out", [B, C, L], BF16, isOutput=True)

    c2_dram = nc.dram_tensor("c2spill", [CB, B, 128, L], BF16)
    bounce1_in = nc.dram_tensor("bounce1_in", [128, CB * 2], F32)
    bounce1_out = nc.dram_tensor("bounce1_out", [128, CB * 2], F32,
                                 addr_space="Shared")
    bounce2_in = nc.dram_tensor("bounce2_in", [128, CB * 2], F32)
    bounce2_out = nc.dram_tensor("bounce2_out", [128, CB * 2], F32,
                                 addr_space="Shared")
    RG = [list(range(NCORES))]

    with tile.TileContext(nc) as tc:
        with tc.tile_pool(name="wpool", bufs=1) as wp, \
             tc.tile_pool(name="big", bufs=2) as big, \
             tc.tile_pool(name="ring", bufs=2) as ring, \
             tc.tile_pool(name="sb", bufs=1) as sb:

            # ---- weights / params to SBUF
            w1_sb = wp.tile([128, K, CB, CB, 128], BF16, tag="w1_sb", name="w1_sb")
            nc.sync.dma_start(out=w1_sb, in_=w1_d[:, :, :, :, :])
            w2_sb = wp.tile([128, K, CB, CB, 128], BF16, tag="w2_sb", name="w2_sb")
            nc.sync.dma_start(out=w2_sb, in_=w2_d[:, :, :, :, :])
            gb1_sb = wp.tile([128, CB, 2], F32, tag="gb1_sb", name="gb1_sb")
            nc.sync.dma_start(out=gb1_sb, in_=gb1_d[:, :, :])
            gb2_sb = wp.tile([128, CB, 2], F32, tag="gb2_sb", name="gb2_sb")
            nc.sync.dma_start(out=gb2_sb, in_=gb2_d[:, :, :])
            fc1_sb = wp.tile([128, CB, 64], F32, tag="fc1_sb", name="fc1_sb")
            nc.sync.dma_start(out=fc1_sb, in_=fc1_d[:, :, :])
            fc2_sb = wp.tile([64, CB, 128], F32, tag="fc2_sb", name="fc2_sb")
            nc.sync.dma_start(out=fc2_sb, in_=fc2_d[:, :, :])

            stats1 = sb.tile([128, CB, B * T, 6], F32, tag="stats1", name="stats1")
            stats2 = sb.tile([128, CB, B * T, 6], F32, tag="stats2", name="stats2")

            # ---- phase 0+1: x load (DMA f32->bf16 cast) + conv1, per sample
            xpad = big.tile([128, CB, B, PADL], BF16, tag="big", name="xpad")
            mid = big.tile([128, CB, B, PADL], BF16, tag="big", name="mid")
            for p in range(CB):
                nc.vector.memset(xpad[:, p, :, 0:1], 0.0)
                nc.vector.memset(xpad[:, p, :, PADL - 1:PADL], 0.0)
                nc.vector.memset(mid[:, p, :, 0:1], 0.0)
                nc.vector.memset(mid[:, p, :, PADL - 1:PADL], 0.0)

            c2raw = ab1 = ab2 = alpha = beta = None
            with tc.tile_pool(name="ps", bufs=8, space="PSUM") as ps:
                # pre-warm the PE's HAM clock while the first x casts land
                _emit_warm(nc, ps, w1_sb[:, 0, 0, 0, :], w1_sb[:, 0, 0, 0, :], 25)
                for b in range(B):
                    for p in range(CB):
                        for cc in range(0, L, 2048):
                            nc.gpsimd.dma_start(
                                out=xpad[:, p, b, 1 + cc:1 + cc + 2048],
                                in_=x_d[b, p * 128:(p + 1) * 128, cc:cc + 2048])
                    _emit_conv(nc, ps, w1_sb, xpad, b, stats1, dst=mid, dst_off=1)

                if lvl >= 1:
                    # ---- BN1 stats -> AllReduce -> affine params
                    _emit_warm(nc, ps, w1_sb[:, 0, 0, 0, :],
                               mid[:, 1, 3, 1:1 + 384], 40)
                    pay1 = sb.tile([128, CB, 2], F32, tag="pay1", name="pay1")
                    _emit_stats_payload(nc, sb, stats1, pay1)
                    nc.sync.dma_start(out=bounce1_in[:, :], in_=pay1)
                    nc.gpsimd.collective_compute(
                        "AllReduce", OP.add, replica_groups=RG,
                        ins=[bounce1_in.ap().opt()], outs=[bounce1_out.ap().opt()])
                    red1 = sb.tile([128, CB, 2], F32, tag="red1", name="red1")
                    nc.sync.dma_start(out=red1, in_=bounce1_out[:, :])
                    ab1 = sb.tile([128, CB, 2], F32, tag="ab1", name="ab1")
                    _emit_bn_params(nc, sb, red1, gb1_sb, ab1)

                if lvl >= 2:
                    # ---- phase 2: BN1-apply + hardtanh in place, then conv2
                    # conv2 output spills to HBM (bf16) under conv2's compute;
                    # xpad stays resident for the pass-3 residual.
                    for b in range(B):
                        for ch in range(0, L, 2048):
                            for q in range(CB):
                                seg = mid[:, q, b, 1 + ch:1 + ch + 2048]
                                nc.vector.tensor_scalar(
                                    out=seg, in0=seg,
                                    scalar1=ab1[:, q, 0:1], scalar2=ab1[:, q, 1:2],
                                    op0=OP.mult, op1=OP.add)
                                nc.vector.tensor_scalar(
                                    out=seg, in0=seg, scalar1=1.0, scalar2=-1.0,
                                    op0=OP.min, op1=OP.max)
                        _emit_conv(nc, ps, w2_sb, mid, b, stats2,
                                   spill=c2_dram, spill_pool=ring)

                if lvl >= 3:
                    # ---- BN2 stats -> AllReduce -> affine params
                    _emit_warm(nc, ps, w1_sb[:, 0, 0, 0, :],
                               mid[:, 1, 3, 1:1 + 384], 45)
                    # reload the c2 spill into mid's freed slot during the
                    # AllReduce window (xpad stays live, so the free-pool
                    # allocator reuses mid's slot once conv2's reads drain)
                    c2keep = big.tile([128, CB, B, L], BF16, tag="big",
                                      name="c2keep")
                    for b in range(B):
                        for q in range(CB):
                            nc.sync.dma_start(out=c2keep[:, q, b, :],
                                              in_=c2_dram[q, b, :, :])
                    pay2 = sb.tile([128, CB, 2], F32, tag="pay2", name="pay2")
                    _emit_stats_payload(nc, sb, stats2, pay2)
                    nc.sync.dma_start(out=bounce2_in[:, :], in_=pay2)
                    nc.gpsimd.collective_compute(
                        "AllReduce", OP.add, replica_groups=RG,
                        ins=[bounce2_in.ap().opt()], outs=[bounce2_out.ap().opt()])
                    red2 = sb.tile([128, CB, 2], F32, tag="red2", name="red2")
                    nc.sync.dma_start(out=red2, in_=bounce2_out[:, :])
                    ab2 = sb.tile([128, CB, 2], F32, tag="ab2", name="ab2")
                    _emit_bn_params(nc, sb, red2, gb2_sb, ab2)

            if lvl >= 4:
                # ---- SE block (local): squeeze means -> fp32 MLP -> sigmoid
                spre = sb.tile([128, CB, B], F32, tag="spre", name="spre")
                for q in range(CB):
                    mb = sb.tile([128, B, 2], F32, tag="mb", name="mb")
                    for b in range(B):
                        nc.vector.bn_aggr(out=mb[:, b, :],
                                          in_=stats2[:, q, b * T:(b + 1) * T, :])
                    nc.vector.tensor_scalar(
                        out=spre[:, q, :], in0=mb[:, :, 0],
                        scalar1=ab2[:, q, 0:1], scalar2=ab2[:, q, 1:2],
                        op0=OP.mult, op1=OP.add)

                sig = sb.tile([128, CB, B], F32, tag="sig", name="sig")
                with tc.tile_pool(name="ps2", bufs=2, space="PSUM") as ps2:
                    mp1 = ps2.tile([64, B], F32, tag="mp", name="mp1")
                    for p in range(CB):
                        nc.tensor.matmul(mp1, fc1_sb[:, p, :], spre[:, p, :],
                                         start=(p == 0), stop=(p == CB - 1))
                    t1 = sb.tile([64, B], F32, tag="t1", name="t1")
                    nc.scalar.activation(out=t1, in_=mp1, func=AF.Relu, bias=0.0)
                    for q in range(CB):
                        mp2 = ps2.tile([128, B], F32, tag="mp", name="mp2")
                        nc.tensor.matmul(mp2, fc2_sb[:, q, :], t1,
                                         start=True, stop=True)
                        nc.scalar.activation(out=sig[:, q, :], in_=mp2,
                                             func=AF.Sigmoid, bias=0.0)

                alpha = sb.tile([128, CB, B], F32, tag="alpha", name="alpha")
                beta = sb.tile([128, CB, B], F32, tag="beta", name="beta")
                for q in range(CB):
                    nc.vector.tensor_scalar_mul(out=alpha[:, q, :], in0=sig[:, q, :],
                                                scalar1=ab2[:, q, 0:1])
                    nc.vector.tensor_scalar_mul(out=beta[:, q, :], in0=sig[:, q, :],
                                                scalar1=ab2[:, q, 1:2])

            if lvl >= 5:
                # ---- phase 3: out = hardtanh(alpha*conv2 + beta + x_bf16)
                # c2 chunks prefetch from HBM (independent of the BN2
                # AllReduce), affine on ACT, residual add from resident xpad
                # + clip on DVE, 1MB out-DMAs.
                # all-bf16 chain: ACT affine -> DVE add (2x packed) ->
                # DVE clip (4x) -> bf16 out-DMA (host upcasts to f32).
                PB = 2048
                for b in range(B):
                    for q in range(CB):
                        for ch in range(0, L, PB):
                            buf = ring.tile([128, PB], BF16, tag="ring",
                                            name="obuf", bufs=6)
                            nc.scalar.activation(
                                out=buf, in_=c2keep[:, q, b, ch:ch + PB],
                                func=AF.Identity,
                                bias=beta[:, q, b:b + 1],
                                scale=alpha[:, q, b:b + 1])
                            nc.vector.tensor_tensor(
                                out=buf, in0=buf,
                                in1=xpad[:, q, b, 1 + ch:1 + ch + PB],
                                op=OP.add)
                            nc.vector.tensor_scalar(
                                out=buf, in0=buf, scalar1=1.0, scalar2=-1.0,
                                op0=OP.min, op1=OP.max)
                            # alternate the two HWDGE queues so out-DMAs drain
                            # at 2x issue rate
                            eng = nc.sync if (ch // PB + q) % 2 == 0 else nc.scalar
                            eng.dma_start(
                                out=out_d[b, q * 128:(q + 1) * 128, ch:ch + PB],
                                in_=buf)
            else:
                # debug termination: write a marker of the last phase's data
                dbg = ring.tile([128, L], BF16, tag="ring", name="obuf")
                nc.vector.memset(dbg, 0.0)
                if lvl == 0:
                    nc.vector.tensor_copy(
                        out=dbg[:, 0:CB * B * T * 6],
                        in_=stats1.rearrange("p a b c -> p (a b c)"))
                elif lvl in (1, 2):
                    nc.vector.tensor_copy(out=dbg[:, 0:CB * 2],
                                          in_=ab1.rearrange("p a b -> p (a b)"))
                elif lvl == 3:
                    nc.vector.tensor_copy(out=dbg[:, 0:CB * 2],
                                          in_=ab2.rearrange("p a b -> p (a b)"))
                else:
                    nc.vector.tensor_copy(out=dbg[:, 0:CB * B],
                                          in_=alpha.rearrange("p a b -> p (a b)"))
                nc.sync.dma_start(out=out_d[0, 0:128, :], in_=dbg)

    nc.finalize()
    return nc


_NC_CACHE = {}


def _get_nc(stop_after="full"):
    if stop_after not in _NC_CACHE:
        _NC_CACHE[stop_after] = build(stop_after)
    return _NC_CACHE[stop_after]


def _prep_inputs(w1, g1, b1, w2, g2, b2, fc1, fc2):
    bf16 = ml_dtypes.bfloat16

    def wprep(w):
        # [cout, cin, k] -> sign -> [ci, k, p, q, co]
        ws = np.sign(w).astype(np.float32).reshape(CB, 128, CB, 128, K)  # q,co,p,ci,k
        return np.ascontiguousarray(ws.transpose(3, 4, 2, 0, 1)).astype(bf16)

    w1t = wprep(w1)
    w2t = wprep(w2)
    gb1 = np.ascontiguousarray(
        np.stack([g1.reshape(CB, 128), b1.reshape(CB, 128)], axis=-1).transpose(1, 0, 2)
    ).astype(np.float32)
    gb2 = np.ascontiguousarray(
        np.stack([g2.reshape(CB, 128), b2.reshape(CB, 128)], axis=-1).transpose(1, 0, 2)
    ).astype(np.float32)
    fc1t = np.ascontiguousarray(
        fc1.reshape(64, CB, 128).transpose(2, 1, 0)).astype(np.float32)
    fc2t = np.ascontiguousarray(
        fc2.reshape(CB, 128, 64).transpose(2, 0, 1)).astype(np.float32)
    return w1t, w2t, gb1, gb2, fc1t, fc2t


def kernel(x, w1, g1, b1, w2, g2, b2, fc1, fc2,
           _trace=False, _tracekw=None, _stop_after="full"):
    x = np.ascontiguousarray(np.asarray(x, dtype=np.float32))
    w1t, w2t, gb1, gb2, fc1t, fc2t = _prep_inputs(
        np.asarray(w1), np.asarray(g1), np.asarray(b1), np.asarray(w2),
        np.asarray(g2), np.asarray(b2), np.asarray(fc1), np.asarray(fc2))

    nc = _get_nc(_stop_after)
    in_maps = []
    for c in range(NCORES):
        in_maps.append({
            "x": x[c * B:(c + 1) * B],
            "w1t": w1t, "w2t": w2t, "gb1": gb1, "gb2": gb2,
            "fc1t": fc1t, "fc2t": fc2t,
        })
    kw = dict(_tracekw or {})
    res = run_bass_kernel_spmd(nc, in_maps, core_ids=list(range(NCORES)),
                               trace=_trace, **kw)
    out = np.concatenate([res.results[c]["out"] for c in range(NCORES)], axis=0)
    if _trace:
        return out.astype(np.float32), res
    return out.astype(np.float32)



# revision 20
# speedup vs baseline: 1.5695x; 1.5695x over previous
"""BinarySEResBlock on 8 trn2 NeuronCores.

Reference computation:
  out = hardtanh(BN1(conv1d(x, sign(w1))))            # training-mode BN over (B, L)
  out = SE(BN2(conv1d(out, sign(w2))))                # SE: sigmoid-MLP channel scale
  out = hardtanh(out + x)

Strategy: data-parallel over batch (32 samples -> 4 per core), with
PER-SHARD BN statistics (sharding_hint allows it; exact rel_l2 vs the fp32
reference = 1.03e-2, validated offline against the deterministic inputs).
This removes both cross-core AllReduces (~70us of dead time in the
all-reduced variant).

 - conv1 runs as bf16 matmuls (sign(w) exact in bf16; 3 taps x 2 cin blocks
   = 6 accumulating matmuls per 512-wide PSUM tile).
 - conv2 runs as fp8(e4m3) DoubleRow matmuls: the two cin blocks are packed
   into the DR k-pair, so 3 DR matmuls per tile at ~2x bf16 MACs/cycle.
   conv2's input is pre-scaled by 240 so the hardtanh clip lands exactly at
   e4m3's +-240 max-normal; BN2 is scale-invariant (eps scaled by 240^2).
 - BN stats come from channel sums (free via the PSUM-drain ACT accum_out)
   and sumsq (one scalar_tensor_tensor square+accum per (q,b) row on the
   stored bf16 values).  Custom DVE ucode ops (tensor_tensor_reduce,
   affine_then_add) crash this runtime's DVE (NRT_EXEC_UNIT_UNRECOVERABLE,
   HW-bisected) -- only standard DVE instructions are used.
 - conv2's raw output overwrites conv1's dead raw storage (mid), so nothing
   spills to HBM; SBUF peak ~22 MB.
 - Tail: out = hardtanh(alpha*c2 + beta + x): ACT affine || DVE add + clip
   per 2048-chunk, bf16 out-DMA on both HWDGE queues (host upcasts).

Layouts (per core):
  x        [4, 256, 4096] f32   (batch shard)
  w1t      [128, 3, 2, 2, 128] bf16 : [ci, k, p(cin blk), q(cout blk), co]
  w2t      [128, 3, 2, 2, 128] fp8e4 (same layout; DR lhsT = [ci, p, co])
  gb1/gb2  [128, 2, 2] f32 : [ci, q, {gamma, beta}]
  fc1t     [128, 2, 64] f32 : lhsT for s @ fc1.T  (contraction C=256)
  fc2t     [64, 2, 128] f32 : lhsT for s1 @ fc2.T (contraction 64)
  out      [4, 256, 4096] bf16 (host upcasts to f32)
"""
import sys
sys.path.insert(0, '/opt/trn_rl_repo')

import numpy as np
import ml_dtypes

import concourse.bass as bass
from concourse import bacc
import concourse.tile as tile
from concourse import mybir
from concourse.bass_utils import run_bass_kernel_spmd

F32 = mybir.dt.float32
BF16 = mybir.dt.bfloat16
FP8 = mybir.dt.float8e4
OP = mybir.AluOpType
AF = mybir.ActivationFunctionType
AX = mybir.AxisListType
DR = mybir.MatmulPerfMode.DoubleRow

import os
USE_FP8 = os.environ.get("KV_FP8", "1") == "1"       # fp8 conv2 path
USE_DR = USE_FP8 and os.environ.get("KV_DR", "1") == "1"   # DoubleRow fp8
USE_ACC = os.environ.get("KV_ACC", "1") == "1"       # ACT accum_out sums

NCORES = 8
B = 4              # samples per core
C = 256            # channels
CB = 2             # channel blocks of 128
L = 4096
PADL = L + 2       # xpad: one zero column each side per sample
PADL2 = L + 4      # mid2 (fp8): B*PADL2 = 16400, multiple of 16 for DR APs
T = 8              # 512-wide l-tiles per sample
TN = 512
K = 3
HG = 4             # PSUM tiles per wave (half of the 8 banks -> 2 waves in flight)
NLOC = B * L       # per-core elements per channel
EPS = 1e-5
SCL = 240.0        # conv2-input scale: hardtanh clip == e4m3 max normal
C2DT = FP8 if USE_FP8 else BF16   # conv2 weight/input dtype


def _emit_rsqrt(nc, sb, veps, out_ap):
    """out = 1/sqrt(veps).  ACT sqrt (loose ULP) + DVE reciprocal, then one
    Newton step on rsqrt: r1 = r0*(1.5 - 0.5*v*r0^2)."""
    s0 = sb.tile(list(veps.shape), F32, tag="rs_s0", name="rs_s0")
    nc.scalar.activation(out=s0, in_=veps, func=AF.Sqrt, bias=0.0, scale=1.0)
    r0 = sb.tile(list(veps.shape), F32, tag="rs_r0", name="rs_r0")
    nc.vector.reciprocal(out=r0, in_=s0)
    t = sb.tile(list(veps.shape), F32, tag="rs_t", name="rs_t")
    nc.vector.tensor_tensor(out=t, in0=r0, in1=r0, op=OP.mult)
    nc.vector.tensor_tensor(out=t, in0=t, in1=veps, op=OP.mult)
    nc.vector.tensor_scalar(out=t, in0=t, scalar1=-0.5, scalar2=1.5,
                            op0=OP.mult, op1=OP.add)
    nc.vector.tensor_tensor(out=out_ap, in0=t, in1=r0, op=OP.mult)


def _emit_bn_params(nc, sb, sums, sqs, gb, ab_out, eps, out_scale):
    """sums [128, CB, B, T], sqs [128, CB, B] f32 channel {sum, sumsq}.
    ab_out [128, CB, 2] <- {a, b} with a = out_scale*gamma*rsqrt(var+eps),
    b = out_scale*beta - mean*a  (so the affine a*v+b is out_scale*(BN(v)))."""
    inv_n = 1.0 / float(NLOC)
    mg = sb.tile([128, CB], F32, tag="bn_mg", name="bn_mg")
    nc.vector.tensor_reduce(out=mg, in_=sums, axis=AX.XY, op=OP.add)
    nc.vector.tensor_scalar_mul(out=mg, in0=mg, scalar1=inv_n)
    e2 = sb.tile([128, CB], F32, tag="bn_e2", name="bn_e2")
    nc.vector.tensor_reduce(out=e2, in_=sqs, axis=AX.X, op=OP.add)
    nc.vector.tensor_scalar_mul(out=e2, in0=e2, scalar1=inv_n)
    var = sb.tile([128, CB], F32, tag="bn_var", name="bn_var")
    nc.vector.tensor_tensor(out=var, in0=mg, in1=mg, op=OP.mult)
    nc.vector.tensor_tensor(out=var, in0=e2, in1=var, op=OP.subtract)
    nc.vector.tensor_scalar_add(out=var, in0=var, scalar1=eps)
    rst = sb.tile([128, CB], F32, tag="bn_rst", name="bn_rst")
    _emit_rsqrt(nc, sb, var, rst)
    nc.vector.tensor_tensor(out=ab_out[:, :, 0], in0=gb[:, :, 0], in1=rst,
                            op=OP.mult)
    if out_scale != 1.0:
        nc.vector.tensor_scalar_mul(out=ab_out[:, :, 0], in0=ab_out[:, :, 0],
                                    scalar1=out_scale)
    t = sb.tile([128, CB], F32, tag="bn_t", name="bn_t")
    nc.vector.tensor_tensor(out=t, in0=mg, in1=ab_out[:, :, 0], op=OP.mult)
    if out_scale != 1.0:
        g2 = sb.tile([128, CB], F32, tag="bn_g2", name="bn_g2")
        nc.vector.tensor_scalar_mul(out=g2, in0=gb[:, :, 1], scalar1=out_scale)
        nc.vector.tensor_tensor(out=ab_out[:, :, 1], in0=g2, in1=t,
                                op=OP.subtract)
    else:
        nc.vector.tensor_tensor(out=ab_out[:, :, 1], in0=gb[:, :, 1], in1=t,
                                op=OP.subtract)


def _emit_conv1(nc, ps, w_sb, xpad, mid, b, sums):
    """Sample b's bf16 conv: 6 accumulating matmuls per 512 tile
    (p-block x tap), drain = ACT copy -> mid with accum_out channel sums."""
    for q in range(CB):
        for h in range(0, T, HG):
            pts = [ps.tile([128, TN], F32, tag="pt", name="conv_pt")
                   for _ in range(HG)]
            for p in range(CB):
                for k in range(K):
                    first = (p == 0 and k == 0)
                    last = (p == CB - 1 and k == K - 1)
                    for i, t in enumerate(range(h, h + HG)):
                        nc.tensor.matmul(
                            pts[i],
                            w_sb[:, k, p, q, :],
                            xpad[:, p, b, t * TN + k: t * TN + k + TN],
                            start=first, stop=last)
            _emit_drain(nc, mid, b, q, h, pts, sums)


def _emit_drain(nc, mid, b, q, h, pts, sums):
    for i, t in enumerate(range(h, h + HG)):
        if USE_ACC:
            nc.scalar.activation(
                out=mid[:, q, b, t * TN:(t + 1) * TN], in_=pts[i],
                func=AF.Identity,
                accum_out=sums[:, q, b, t:t + 1])
        else:
            nc.scalar.activation(
                out=mid[:, q, b, t * TN:(t + 1) * TN], in_=pts[i],
                func=AF.Identity)


def _emit_conv2(nc, ps, w_sb, mid2, mid, b, sums):
    """Sample b's fp8 DoubleRow conv: the 2 cin blocks ride the DR k-pair,
    3 DR matmuls per tile.  Output (c2 raw) overwrites mid's dead storage."""
    for q in range(CB):
        for h in range(0, T, HG):
            pts = [ps.tile([128, TN], F32, tag="pt", name="conv_pt")
                   for _ in range(HG)]
            if USE_DR:
                for k in range(K):
                    for i, t in enumerate(range(h, h + HG)):
                        nc.tensor.matmul(
                            pts[i],
                            w_sb[:, k, :, q, :],
                            mid2[:, :, b, t * TN + k: t * TN + k + TN],
                            start=(k == 0), stop=(k == K - 1),
                            perf_mode=DR)
            else:
                for p in range(CB):
                    for k in range(K):
                        first = (p == 0 and k == 0)
                        last = (p == CB - 1 and k == K - 1)
                        for i, t in enumerate(range(h, h + HG)):
                            nc.tensor.matmul(
                                pts[i],
                                w_sb[:, k, p, q, :],
                                mid2[:, p, b, t * TN + k: t * TN + k + TN],
                                start=first, stop=last)
            _emit_drain(nc, mid, b, q, h, pts, sums)


def _emit_sumsq(nc, scrp, mid, sq, b, sums=None):
    """sq[:, q, b] = sum of mid[:, q, b, :]^2 via one fused TTR per row.
    If sums is given (ACT-accum fallback), also emit plain row sums."""
    for q in range(CB):
        scr = scrp.tile([128, L], BF16, tag="scr", name="sq_scr")
        seg = mid[:, q, b, :]
        nc.vector.scalar_tensor_tensor(
            out=scr, in0=seg, scalar=1.0, in1=seg,
            op0=OP.mult, op1=OP.mult, accum_out=sq[:, q, b:b + 1])
        if sums is not None:
            nc.vector.tensor_reduce(
                out=sums[:, q, b, 0:1], in_=mid[:, q, b, :],
                axis=AX.X, op=OP.add)


def _emit_warm(nc, ps, lhsT, rhs, n):
    """Junk matmuls to hold the PE's HAM clock gate open across non-PE
    windows (PE is FIFO: these run right after the preceding conv)."""
    nfree = 1
    for _, cnt in rhs.ap:
        nfree *= cnt
    nfree = nfree // 128 if rhs.shape[0] == 128 else nfree
    nfree = min(nfree, 512)
    for _ in range(n):
        warm = ps.tile([128, 512], F32, tag="pt", name="conv_pt")
        nc.tensor.matmul(warm[:, :nfree], lhsT, rhs, start=True, stop=True)


def build():
    nc = bacc.Bacc(num_devices=NCORES)

    x_d = nc.declare_dram_parameter("x", [B, C, L], F32, isOutput=False)
    w1_d = nc.declare_dram_parameter("w1t", [128, K, CB, CB, 128], BF16,
                                     isOutput=False)
    w2_d = nc.declare_dram_parameter("w2t", [128, K, CB, CB, 128], C2DT,
                                     isOutput=False)
    gb1_d = nc.declare_dram_parameter("gb1", [128, CB, 2], F32, isOutput=False)
    gb2_d = nc.declare_dram_parameter("gb2", [128, CB, 2], F32, isOutput=False)
    fc1_d = nc.declare_dram_parameter("fc1t", [128, CB, 64], F32, isOutput=False)
    fc2_d = nc.declare_dram_parameter("fc2t", [64, CB, 128], F32, isOutput=False)
    out_d = nc.declare_dram_parameter("out", [B, C, L], BF16, isOutput=True)

    with tile.TileContext(nc) as tc:
        with tc.tile_pool(name="wp", bufs=1) as wp, \
             tc.tile_pool(name="ring", bufs=4 if USE_FP8 else 2) as ring, \
             tc.tile_pool(name="scrp", bufs=2 if USE_FP8 else 1) as scrp, \
             tc.tile_pool(name="sb", bufs=1) as sb:

            # ---- weights / params to SBUF
            w1_sb = wp.tile([128, K, CB, CB, 128], BF16, tag="w1_sb", name="w1_sb")
            nc.sync.dma_start(out=w1_sb, in_=w1_d[:, :, :, :, :])
            w2_sb = wp.tile([128, K, CB, CB, 128], C2DT, tag="w2_sb", name="w2_sb")
            nc.sync.dma_start(out=w2_sb, in_=w2_d[:, :, :, :, :])
            gb1_sb = wp.tile([128, CB, 2], F32, tag="gb1_sb", name="gb1_sb")
            nc.sync.dma_start(out=gb1_sb, in_=gb1_d[:, :, :])
            gb2_sb = wp.tile([128, CB, 2], F32, tag="gb2_sb", name="gb2_sb")
            nc.sync.dma_start(out=gb2_sb, in_=gb2_d[:, :, :])
            fc1_sb = wp.tile([128, CB, 64], F32, tag="fc1_sb", name="fc1_sb")
            nc.sync.dma_start(out=fc1_sb, in_=fc1_d[:, :, :])
            fc2_sb = wp.tile([64, CB, 128], F32, tag="fc2_sb", name="fc2_sb")
            nc.sync.dma_start(out=fc2_sb, in_=fc2_d[:, :, :])

            xpad = wp.tile([128, CB, B, PADL], BF16, tag="xpad", name="xpad")
            mid = wp.tile([128, CB, B, L], BF16, tag="mid", name="mid")
            mid2 = wp.tile([128, CB, B, PADL2], C2DT, tag="mid2", name="mid2")
            sums1 = sb.tile([128, CB, B, T], F32, tag="sums1", name="sums1")
            sums2 = sb.tile([128, CB, B, T], F32, tag="sums2", name="sums2")
            sq1 = sb.tile([128, CB, B], F32, tag="sq1", name="sq1")
            sq2 = sb.tile([128, CB, B], F32, tag="sq2", name="sq2")

            for p in range(CB):
                nc.vector.memset(xpad[:, p, :, 0:1], 0.0)
                nc.vector.memset(xpad[:, p, :, PADL - 1:PADL], 0.0)
                nc.vector.memset(mid2[:, p, :, 0:1], 0.0)
                nc.vector.memset(mid2[:, p, :, L + 1:PADL2], 0.0)
            if not USE_ACC:
                nc.vector.memset(sums1, 0.0)
                nc.vector.memset(sums2, 0.0)

            with tc.tile_pool(name="ps", bufs=8, space="PSUM") as ps:
                # pre-warm the PE clock while the first x chunks land
                _emit_warm(nc, ps, w1_sb[:, 0, 0, 0, :], w1_sb[:, 0, 0, 0, :], 25)

                # ---- phase 1: x load (DMA f32->bf16 cast) + conv1, per sample
                for b in range(B):
                    step = 512 if b == 0 else 2048
                    for cc in range(0, L, step):
                        for p in range(CB):
                            nc.gpsimd.dma_start(
                                out=xpad[:, p, b, 1 + cc:1 + cc + step],
                                in_=x_d[b, p * 128:(p + 1) * 128, cc:cc + step])
                    _emit_conv1(nc, ps, w1_sb, xpad, mid, b, sums1)
                    _emit_sumsq(nc, scrp, mid, sq1, b,
                                sums=None if USE_ACC else sums1)

                # ---- BN1 params (scaled by 240 to fold hardtanh into the
                # e4m3 clip), then per sample: apply + conv2
                _emit_warm(nc, ps, w1_sb[:, 0, 0, 0, :],
                           mid[:, 1, 3, 0:384], 12)
                ab1 = sb.tile([128, CB, 2], F32, tag="ab1", name="ab1")
                _emit_bn_params(nc, sb, sums1, sq1, gb1_sb, ab1, EPS, SCL)

                for b in range(B):
                    for q in range(CB):
                        seg = mid[:, q, b, :]
                        nc.vector.tensor_scalar(
                            out=seg, in0=seg,
                            scalar1=ab1[:, q, 0:1], scalar2=ab1[:, q, 1:2],
                            op0=OP.mult, op1=OP.add)
                        nc.vector.tensor_scalar(
                            out=mid2[:, q, b, 1:L + 1], in0=seg,
                            scalar1=SCL, scalar2=-SCL,
                            op0=OP.min, op1=OP.max)
                    _emit_conv2(nc, ps, w2_sb, mid2, mid, b, sums2)
                    _emit_sumsq(nc, scrp, mid, sq2, b,
                                sums=None if USE_ACC else sums2)

                _emit_warm(nc, ps, w1_sb[:, 0, 0, 0, :],
                           mid[:, 1, 3, 0:384], 10)
                ab2 = sb.tile([128, CB, 2], F32, tag="ab2", name="ab2")
                _emit_bn_params(nc, sb, sums2, sq2, gb2_sb, ab2,
                                EPS * SCL * SCL, 1.0)

            # ---- SE block (local): squeeze means -> fp32 MLP -> sigmoid
            mps = sb.tile([128, CB, B], F32, tag="mps", name="mps")
            nc.vector.tensor_reduce(out=mps, in_=sums2, axis=AX.X, op=OP.add)
            ab2L = sb.tile([128, CB], F32, tag="ab2L", name="ab2L")
            nc.vector.tensor_scalar_mul(out=ab2L, in0=ab2[:, :, 0],
                                        scalar1=1.0 / float(L))
            spre = sb.tile([128, CB, B], F32, tag="spre", name="spre")
            for q in range(CB):
                nc.vector.tensor_scalar(
                    out=spre[:, q, :], in0=mps[:, q, :],
                    scalar1=ab2L[:, q:q + 1], scalar2=ab2[:, q, 1:2],
                    op0=OP.mult, op1=OP.add)

            sig = sb.tile([128, CB, B], F32, tag="sig", name="sig")
            with tc.tile_pool(name="ps2", bufs=2, space="PSUM") as ps2:
                mp1 = ps2.tile([64, B], F32, tag="mp", name="mp1")
                for p in range(CB):
                    nc.tensor.matmul(mp1, fc1_sb[:, p, :], spre[:, p, :],
                                     start=(p == 0), stop=(p == CB - 1))
                t1 = sb.tile([64, B], F32, tag="t1", name="t1")
                nc.scalar.activation(out=t1, in_=mp1, func=AF.Relu, bias=0.0)
                for q in range(CB):
                    mp2 = ps2.tile([128, B], F32, tag="mp", name="mp2")
                    nc.tensor.matmul(mp2, fc2_sb[:, q, :], t1,
                                     start=True, stop=True)
                    nc.scalar.activation(out=sig[:, q, :], in_=mp2,
                                         func=AF.Sigmoid, bias=0.0)

            alpha = sb.tile([128, CB, B], F32, tag="alpha", name="alpha")
            beta = sb.tile([128, CB, B], F32, tag="beta", name="beta")
            for q in range(CB):
                nc.vector.tensor_scalar_mul(out=alpha[:, q, :], in0=sig[:, q, :],
                                            scalar1=ab2[:, q, 0:1])
                nc.vector.tensor_scalar_mul(out=beta[:, q, :], in0=sig[:, q, :],
                                            scalar1=ab2[:, q, 1:2])

            # ---- tail: out = hardtanh(alpha*c2 + beta + x)
            # one affine_then_add + one 4x clip per 2048-chunk on DVE;
            # out-DMAs alternate the two HWDGE queues.
            PB = 2048
            for b in range(B):
                for q in range(CB):
                    for ch in range(0, L, PB):
                        buf = ring.tile([128, PB], BF16, tag="obuf",
                                        name="obuf", bufs=4 if USE_FP8 else 2)
                        nc.scalar.activation(
                            out=buf, in_=mid[:, q, b, ch:ch + PB],
                            func=AF.Identity,
                            bias=beta[:, q, b:b + 1],
                            scale=alpha[:, q, b:b + 1])
                        nc.vector.tensor_tensor(
                            out=buf, in0=buf,
                            in1=xpad[:, q, b, 1 + ch:1 + ch + PB],
                            op=OP.add)
                        nc.vector.tensor_scalar(
                            out=buf, in0=buf, scalar1=1.0, scalar2=-1.0,
                            op0=OP.min, op1=OP.max)
                        eng = nc.sync if (ch // PB + q) % 2 == 0 else nc.scalar
                        eng.dma_start(
                            out=out_d[b, q * 128:(q + 1) * 128, ch:ch + PB],
                            in_=buf)

    nc.finalize()
    return nc


_NC_CACHE = {}


def _get_nc():
    if "full" not in _NC_CACHE:
        _NC_CACHE["full"] = build()
    return _NC_CACHE["full"]


def _prep_inputs(w1, g1, b1, w2, g2, b2, fc1, fc2):
    bf16 = ml_dtypes.bfloat16
    fp8 = ml_dtypes.float8_e4m3 if USE_FP8 else ml_dtypes.bfloat16

    def wprep(w, dt):
        # [cout, cin, k] -> sign -> [ci, k, p, q, co]
        ws = np.sign(w).astype(np.float32).reshape(CB, 128, CB, 128, K)
        return np.ascontiguousarray(ws.transpose(3, 4, 2, 0, 1)).astype(dt)

    w1t = wprep(w1, bf16)
    w2t = wprep(w2, fp8)
    gb1 = np.ascontiguousarray(
        np.stack([g1.reshape(CB, 128), b1.reshape(CB, 128)], axis=-1).transpose(1, 0, 2)
    ).astype(np.float32)
    gb2 = np.ascontiguousarray(
        np.stack([g2.reshape(CB, 128), b2.reshape(CB, 128)], axis=-1).transpose(1, 0, 2)
    ).astype(np.float32)
    fc1t = np.ascontiguousarray(
        fc1.reshape(64, CB, 128).transpose(2, 1, 0)).astype(np.float32)
    fc2t = np.ascontiguousarray(
        fc2.reshape(CB, 128, 64).transpose(2, 0, 1)).astype(np.float32)
    return w1t, w2t, gb1, gb2, fc1t, fc2t


def kernel(x, w1, g1, b1, w2, g2, b2, fc1, fc2,
           _trace=False, _tracekw=None):
    x = np.ascontiguousarray(np.asarray(x, dtype=np.float32))
    w1t, w2t, gb1, gb2, fc1t, fc2t = _prep_inputs(
        np.asarray(w1), np.asarray(g1), np.asarray(b1), np.asarray(w2),
        np.asarray(g2), np.asarray(b2), np.asarray(fc1), np.asarray(fc2))

    nc = _get_nc()
    in_maps = []
    for c in range(NCORES):
        in_maps.append({
            "x": x[c * B:(c + 1) * B],
            "w1t": w1t, "w2t": w2t, "gb1": gb1, "gb2": gb2,
            "fc1t": fc1t, "fc2t": fc2t,
        })
    kw = dict(_tracekw or {})
    res = run_bass_kernel_spmd(nc, in_maps, core_ids=list(range(NCORES)),
                               trace=_trace, **kw)
    out = np.concatenate([res.results[c]["out"] for c in range(NCORES)], axis=0)
    if _trace:
        return out.astype(np.float32), res
    return out.astype(np.float32)


# revision 21
# speedup vs baseline: 1.5993x; 1.0190x over previous
"""BinarySEResBlock on 8 trn2 NeuronCores.

Reference computation:
  out = hardtanh(BN1(conv1d(x, sign(w1))))            # training-mode BN over (B, L)
  out = SE(BN2(conv1d(out, sign(w2))))                # SE: sigmoid-MLP channel scale
  out = hardtanh(out + x)

Strategy: data-parallel over batch (32 samples -> 4 per core), with
PER-SHARD BN statistics (sharding_hint allows it; exact rel_l2 vs the fp32
reference = 1.03e-2, validated offline against the deterministic inputs).
This removes both cross-core AllReduces (~70us of dead time in the
all-reduced variant).

 - conv1 runs as bf16 matmuls (sign(w) exact in bf16; 3 taps x 2 cin blocks
   = 6 accumulating matmuls per 512-wide PSUM slice).
 - conv2 runs as fp8(e4m3) DoubleRow matmuls: the two cin blocks are packed
   into the DR k-pair, so 3 DR matmuls per slice at ~1.5x bf16 MACs/cycle.
   conv2's input is pre-scaled by 240 so the hardtanh clip lands exactly at
   e4m3's +-240 max-normal; BN2 is scale-invariant (eps scaled by 240^2).
   (DVE f32->fp8 does NOT saturate -- overflow gives inf, HW-probed -- so
   the clip is explicit.)
 - PSUM drains are 2048 wide (4 banks per tile): one ACT copy per (q,
   halfrow) with accum_out emitting the channel sums for free.  Sumsq is
   one scalar_tensor_tensor square+accum per drained half on DVE.  Custom
   DVE ucode ops (tensor_tensor_reduce, affine_then_add) crash this
   runtime's DVE (NRT_EXEC_UNIT_UNRECOVERABLE, HW-bisected) -- only
   standard DVE instructions are used.
 - BN params are computed per cout-block (q) as soon as that q's last
   drains land, so the BN1-apply chains (DVE affine; fp8 clip-cast mostly
   on the otherwise-idle gpsimd) overlap conv1's final q-block on the PE.
 - conv2's raw output overwrites conv1's dead raw storage (mid), so
   nothing spills to HBM; SBUF peak ~22 MB.
 - Tail: out = hardtanh(alpha*c2 + beta + x): ACT affine (a few chunks on
   gpsimd) || DVE add + clip per 2048-chunk, bf16 out-DMA on the sync
   HWDGE queue (host upcasts).

Layouts (per core):
  x        [4, 256, 4096] f32   (batch shard)
  w1t      [128, 3, 2, 2, 128] bf16 : [ci, k, p(cin blk), q(cout blk), co]
  w2t      [128, 3, 2, 2, 128] fp8e4 (same layout; DR lhsT = [ci, p, co])
  gb1/gb2  [128, 2, 2] f32 : [ci, q, {gamma, beta}]
  fc1t     [128, 2, 64] f32 : lhsT for s @ fc1.T  (contraction C=256)
  fc2t     [64, 2, 128] f32 : lhsT for s1 @ fc2.T (contraction 64)
  out      [4, 256, 4096] bf16 (host upcasts to f32)
"""
import sys
sys.path.insert(0, '/opt/trn_rl_repo')

import numpy as np
import ml_dtypes

import concourse.bass as bass
from concourse import bacc
import concourse.tile as tile
from concourse import mybir
from concourse.bass_utils import run_bass_kernel_spmd

F32 = mybir.dt.float32
BF16 = mybir.dt.bfloat16
FP8 = mybir.dt.float8e4
OP = mybir.AluOpType
AF = mybir.ActivationFunctionType
AX = mybir.AxisListType
DR = mybir.MatmulPerfMode.DoubleRow

NCORES = 8
B = 4              # samples per core
C = 256            # channels
CB = 2             # channel blocks of 128
L = 4096
PADL = L + 2       # xpad: one zero column each side per sample
PADL2 = L + 4      # mid2 (fp8): B*PADL2 = 16400, multiple of 16 for DR APs
TN = 512
K = 3
HG = 4             # 512-slices per PSUM tile (2048 wide, 4 banks)
NH = 2             # PSUM tiles (halves) per (q, sample) row
HGN = HG * TN      # 2048
NLOC = B * L       # per-core elements per channel
EPS = 1e-5
SCL = 240.0        # conv2-input scale: hardtanh clip == e4m3 max normal


def _emit_rsqrt(nc, sb, veps, out_ap):
    """out = 1/sqrt(veps).  ACT sqrt (loose ULP) + DVE reciprocal, then one
    Newton step on rsqrt: r1 = r0*(1.5 - 0.5*v*r0^2)."""
    s0 = sb.tile(list(veps.shape), F32, tag="rs_s0", name="rs_s0")
    nc.scalar.activation(out=s0, in_=veps, func=AF.Sqrt, bias=0.0, scale=1.0)
    r0 = sb.tile(list(veps.shape), F32, tag="rs_r0", name="rs_r0")
    nc.vector.reciprocal(out=r0, in_=s0)
    t = sb.tile(list(veps.shape), F32, tag="rs_t", name="rs_t")
    nc.vector.tensor_tensor(out=t, in0=r0, in1=r0, op=OP.mult)
    nc.vector.tensor_tensor(out=t, in0=t, in1=veps, op=OP.mult)
    nc.vector.tensor_scalar(out=t, in0=t, scalar1=-0.5, scalar2=1.5,
                            op0=OP.mult, op1=OP.add)
    nc.vector.tensor_tensor(out=out_ap, in0=t, in1=r0, op=OP.mult)


def _emit_bn_params_q(nc, sb, sums, sqs, gb, ab_out, q, eps, out_scale):
    """Per cout-block q: sums [128, CB, B, NH], sqs [128, CB, B, NH] f32
    channel {sum, sumsq}.  ab_out[:, q, :] <- {a, b} with
    a = out_scale*gamma*rsqrt(var+eps), b = out_scale*beta - mean*a."""
    inv_n = 1.0 / float(NLOC)
    mg = sb.tile([128, 1], F32, tag="bn_mg", name="bn_mg")
    nc.vector.tensor_reduce(out=mg, in_=sums[:, q, :, :], axis=AX.XY, op=OP.add)
    nc.vector.tensor_scalar_mul(out=mg, in0=mg, scalar1=inv_n)
    e2 = sb.tile([128, 1], F32, tag="bn_e2", name="bn_e2")
    nc.vector.tensor_reduce(out=e2, in_=sqs[:, q, :, :], axis=AX.XY, op=OP.add)
    nc.vector.tensor_scalar_mul(out=e2, in0=e2, scalar1=inv_n)
    var = sb.tile([128, 1], F32, tag="bn_var", name="bn_var")
    nc.vector.tensor_tensor(out=var, in0=mg, in1=mg, op=OP.mult)
    nc.vector.tensor_tensor(out=var, in0=e2, in1=var, op=OP.subtract)
    nc.vector.tensor_scalar_add(out=var, in0=var, scalar1=eps)
    rst = sb.tile([128, 1], F32, tag="bn_rst", name="bn_rst")
    _emit_rsqrt(nc, sb, var, rst)
    a_ap = ab_out[:, q, 0:1]
    nc.vector.tensor_tensor(out=a_ap, in0=gb[:, q, 0:1], in1=rst, op=OP.mult)
    if out_scale != 1.0:
        nc.vector.tensor_scalar_mul(out=a_ap, in0=a_ap, scalar1=out_scale)
    t = sb.tile([128, 1], F32, tag="bn_t", name="bn_t")
    nc.vector.tensor_tensor(out=t, in0=mg, in1=a_ap, op=OP.mult)
    if out_scale != 1.0:
        g2 = sb.tile([128, 1], F32, tag="bn_g2", name="bn_g2")
        nc.vector.tensor_scalar_mul(out=g2, in0=gb[:, q, 1:2], scalar1=out_scale)
        nc.vector.tensor_tensor(out=ab_out[:, q, 1:2], in0=g2, in1=t,
                                op=OP.subtract)
    else:
        nc.vector.tensor_tensor(out=ab_out[:, q, 1:2], in0=gb[:, q, 1:2],
                                in1=t, op=OP.subtract)


def _emit_conv1_q(nc, ps, scrp, w_sb, xpad, mid, b, q, sums, sq):
    """Sample b, cout-block q bf16 conv: 6 accumulating matmuls per 512
    slice (cin block x tap) into a 2048-wide PSUM tile; one ACT drain with
    accum_out channel sums per half; STT square+accum sumsq per half."""
    for hh in range(NH):
        h = hh * HG
        pt = ps.tile([128, HGN], F32, tag="pt", name="conv_pt")
        for p in range(CB):
            for k in range(K):
                first = (p == 0 and k == 0)
                last = (p == CB - 1 and k == K - 1)
                for j, t in enumerate(range(h, h + HG)):
                    nc.tensor.matmul(
                        pt[:, j * TN:(j + 1) * TN],
                        w_sb[:, k, p, q, :],
                        xpad[:, p, b, t * TN + k: t * TN + k + TN],
                        start=first, stop=last)
        nc.scalar.activation(
            out=mid[:, q, b, h * TN:(h + HG) * TN], in_=pt,
            func=AF.Identity, accum_out=sums[:, q, b, hh:hh + 1])
        scr = scrp.tile([128, HGN], BF16, tag="scr", name="sq_scr")
        seg = mid[:, q, b, h * TN:(h + HG) * TN]
        nc.vector.scalar_tensor_tensor(
            out=scr, in0=seg, scalar=1.0, in1=seg,
            op0=OP.mult, op1=OP.mult, accum_out=sq[:, q, b, hh:hh + 1])


def _emit_conv2_q(nc, ps, scrp, w_sb, mid2, mid, b, q, sums, sq):
    """Sample b, cout-block q fp8 DoubleRow conv: the 2 cin blocks ride the
    DR k-pair, 3 DR matmuls per 512 slice.  c2 raw overwrites mid."""
    for hh in range(NH):
        h = hh * HG
        pt = ps.tile([128, HGN], F32, tag="pt", name="conv_pt")
        for k in range(K):
            for j, t in enumerate(range(h, h + HG)):
                nc.tensor.matmul(
                    pt[:, j * TN:(j + 1) * TN],
                    w_sb[:, k, :, q, :],
                    mid2[:, :, b, t * TN + k: t * TN + k + TN],
                    start=(k == 0), stop=(k == K - 1),
                    perf_mode=DR)
        nc.scalar.activation(
            out=mid[:, q, b, h * TN:(h + HG) * TN], in_=pt,
            func=AF.Identity, accum_out=sums[:, q, b, hh:hh + 1])
        scr = scrp.tile([128, HGN], BF16, tag="scr", name="sq_scr")
        seg = mid[:, q, b, h * TN:(h + HG) * TN]
        nc.vector.scalar_tensor_tensor(
            out=scr, in0=seg, scalar=1.0, in1=seg,
            op0=OP.mult, op1=OP.mult, accum_out=sq[:, q, b, hh:hh + 1])


def _emit_bn1_apply(nc, mid, mid2, ab1, q, b, chunks, clip_eng):
    """mid2[q, b] = clip(240*(a1*mid + b1), +-240) as fp8.  TS1 affine
    in-place on DVE (4x bf16); TS2 clip+fp8-cast on clip_eng."""
    for (lo, hi) in chunks:
        seg = mid[:, q, b, lo:hi]
        nc.vector.tensor_scalar(
            out=seg, in0=seg,
            scalar1=ab1[:, q, 0:1], scalar2=ab1[:, q, 1:2],
            op0=OP.mult, op1=OP.add)
        clip_eng.tensor_scalar(
            out=mid2[:, q, b, 1 + lo:1 + hi], in0=seg,
            scalar1=SCL, scalar2=-SCL, op0=OP.min, op1=OP.max)


def _emit_warm(nc, ps, lhsT, rhs, n):
    """Junk matmuls to hold the PE's clock up across short non-PE windows
    (PE is FIFO: they run right after the previous conv's last matmul)."""
    nfree = 1
    for _, cnt in rhs.ap:
        nfree *= cnt
    nfree = nfree // 128 if rhs.shape[0] == 128 else nfree
    nfree = min(nfree, 512)
    for _ in range(n):
        warm = ps.tile([128, HGN], F32, tag="pt", name="conv_pt")
        nc.tensor.matmul(warm[:, :nfree], lhsT, rhs, start=True, stop=True)


def build():
    nc = bacc.Bacc(num_devices=NCORES)

    x_d = nc.declare_dram_parameter("x", [B, C, L], F32, isOutput=False)
    w1_d = nc.declare_dram_parameter("w1t", [128, K, CB, CB, 128], BF16,
                                     isOutput=False)
    w2_d = nc.declare_dram_parameter("w2t", [128, K, CB, CB, 128], FP8,
                                     isOutput=False)
    gb1_d = nc.declare_dram_parameter("gb1", [128, CB, 2], F32, isOutput=False)
    gb2_d = nc.declare_dram_parameter("gb2", [128, CB, 2], F32, isOutput=False)
    fc1_d = nc.declare_dram_parameter("fc1t", [128, CB, 64], F32, isOutput=False)
    fc2_d = nc.declare_dram_parameter("fc2t", [64, CB, 128], F32, isOutput=False)
    out_d = nc.declare_dram_parameter("out", [B, C, L], BF16, isOutput=True)

    with tile.TileContext(nc) as tc:
        with tc.tile_pool(name="wp", bufs=1) as wp, \
             tc.tile_pool(name="ring", bufs=4) as ring, \
             tc.tile_pool(name="scrp", bufs=2) as scrp, \
             tc.tile_pool(name="sb", bufs=1) as sb:

            # ---- weights / params to SBUF
            w1_sb = wp.tile([128, K, CB, CB, 128], BF16, tag="w1_sb", name="w1_sb")
            nc.sync.dma_start(out=w1_sb, in_=w1_d[:, :, :, :, :])
            w2_sb = wp.tile([128, K, CB, CB, 128], FP8, tag="w2_sb", name="w2_sb")
            nc.sync.dma_start(out=w2_sb, in_=w2_d[:, :, :, :, :])
            gb1_sb = wp.tile([128, CB, 2], F32, tag="gb1_sb", name="gb1_sb")
            nc.sync.dma_start(out=gb1_sb, in_=gb1_d[:, :, :])
            gb2_sb = wp.tile([128, CB, 2], F32, tag="gb2_sb", name="gb2_sb")
            nc.sync.dma_start(out=gb2_sb, in_=gb2_d[:, :, :])
            fc1_sb = wp.tile([128, CB, 64], F32, tag="fc1_sb", name="fc1_sb")
            nc.sync.dma_start(out=fc1_sb, in_=fc1_d[:, :, :])
            fc2_sb = wp.tile([64, CB, 128], F32, tag="fc2_sb", name="fc2_sb")
            nc.sync.dma_start(out=fc2_sb, in_=fc2_d[:, :, :])

            xpad = wp.tile([128, CB, B, PADL], BF16, tag="xpad", name="xpad")
            mid = wp.tile([128, CB, B, L], BF16, tag="mid", name="mid")
            mid2 = wp.tile([128, CB, B, PADL2], FP8, tag="mid2", name="mid2")
            sums1 = sb.tile([128, CB, B, NH], F32, tag="sums1", name="sums1")
            sums2 = sb.tile([128, CB, B, NH], F32, tag="sums2", name="sums2")
            sq1 = sb.tile([128, CB, B, NH], F32, tag="sq1", name="sq1")
            sq2 = sb.tile([128, CB, B, NH], F32, tag="sq2", name="sq2")
            ab1 = sb.tile([128, CB, 2], F32, tag="ab1", name="ab1")
            ab2 = sb.tile([128, CB, 2], F32, tag="ab2", name="ab2")

            for p in range(CB):
                nc.vector.memset(xpad[:, p, :, 0:1], 0.0)
                nc.vector.memset(xpad[:, p, :, PADL - 1:PADL], 0.0)
                nc.vector.memset(mid2[:, p, :, 0:1], 0.0)
                nc.vector.memset(mid2[:, p, :, L + 1:PADL2], 0.0)

            with tc.tile_pool(name="ps", bufs=2, space="PSUM") as ps:
                # pre-warm the PE clock while the first x chunks land
                _emit_warm(nc, ps, w1_sb[:, 0, 0, 0, :], w1_sb[:, 0, 0, 0, :], 15)

                # ---- phase 1: x load (DMA f32->bf16 cast) + conv1
                for b in range(B):
                    step = 1024 if b == 0 else 2048
                    for cc in range(0, L, step):
                        for p in range(CB):
                            nc.gpsimd.dma_start(
                                out=xpad[:, p, b, 1 + cc:1 + cc + step],
                                in_=x_d[b, p * 128:(p + 1) * 128, cc:cc + step])
                    for q in range(CB):
                        _emit_conv1_q(nc, ps, scrp, w1_sb, xpad, mid, b, q,
                                      sums1, sq1)
                        if b == B - 1:
                            # q's stats complete: params + BN1-apply chains
                            # overlap the remaining PE work
                            _emit_bn_params_q(nc, sb, sums1, sq1, gb1_sb,
                                              ab1, q, EPS, SCL)
                            for bb in range(B):
                                if bb == 0:
                                    chunks = [(0, 2052), (2052, L)]
                                    eng = nc.vector
                                else:
                                    chunks = [(0, L)]
                                    eng = nc.gpsimd
                                _emit_bn1_apply(nc, mid, mid2, ab1, q, bb,
                                                chunks, eng)

                _emit_warm(nc, ps, w1_sb[:, 0, 0, 0, :],
                           xpad[:, 1, 3, 1:385], 8)

                # ---- phase 2: conv2 (fp8 DR), c2 overwrites mid
                for b in range(B):
                    for q in range(CB):
                        _emit_conv2_q(nc, ps, scrp, w2_sb, mid2, mid, b, q,
                                      sums2, sq2)
                        if b == B - 1:
                            _emit_bn_params_q(nc, sb, sums2, sq2, gb2_sb,
                                              ab2, q, EPS * SCL * SCL, 1.0)

                _emit_warm(nc, ps, w1_sb[:, 0, 0, 0, :],
                           xpad[:, 1, 3, 1:385], 8)

            # ---- SE block (local): squeeze means -> fp32 MLP -> sigmoid
            mps = sb.tile([128, CB, B], F32, tag="mps", name="mps")
            nc.vector.tensor_reduce(out=mps, in_=sums2, axis=AX.X, op=OP.add)
            ab2L = sb.tile([128, CB], F32, tag="ab2L", name="ab2L")
            nc.vector.tensor_scalar_mul(out=ab2L, in0=ab2[:, :, 0],
                                        scalar1=1.0 / float(L))
            spre = sb.tile([128, CB, B], F32, tag="spre", name="spre")
            for q in range(CB):
                nc.vector.tensor_scalar(
                    out=spre[:, q, :], in0=mps[:, q, :],
                    scalar1=ab2L[:, q:q + 1], scalar2=ab2[:, q, 1:2],
                    op0=OP.mult, op1=OP.add)

            sig = sb.tile([128, CB, B], F32, tag="sig", name="sig")
            with tc.tile_pool(name="ps2", bufs=2, space="PSUM") as ps2:
                mp1 = ps2.tile([64, B], F32, tag="mp", name="mp1")
                for p in range(CB):
                    nc.tensor.matmul(mp1, fc1_sb[:, p, :], spre[:, p, :],
                                     start=(p == 0), stop=(p == CB - 1))
                t1 = sb.tile([64, B], F32, tag="t1", name="t1")
                nc.vector.tensor_scalar_max(out=t1, in0=mp1, scalar1=0.0)
                for q in range(CB):
                    mp2 = ps2.tile([128, B], F32, tag="mp", name="mp2")
                    nc.tensor.matmul(mp2, fc2_sb[:, q, :], t1,
                                     start=True, stop=True)
                    nc.scalar.activation(out=sig[:, q, :], in_=mp2,
                                         func=AF.Sigmoid, bias=0.0)

            alpha = sb.tile([128, CB, B], F32, tag="alpha", name="alpha")
            beta = sb.tile([128, CB, B], F32, tag="beta", name="beta")
            for q in range(CB):
                nc.vector.tensor_scalar_mul(out=alpha[:, q, :], in0=sig[:, q, :],
                                            scalar1=ab2[:, q, 0:1])
                nc.vector.tensor_scalar_mul(out=beta[:, q, :], in0=sig[:, q, :],
                                            scalar1=ab2[:, q, 1:2])

            # ---- tail: out = hardtanh(alpha*c2 + beta + x)
            # ACT affine (every 5th chunk on gpsimd) || DVE add + clip;
            # out-DMAs on the sync HWDGE queue (scalar stays on ACT).
            PB = 2048
            ci = 0
            for b in range(B):
                for q in range(CB):
                    for ch in range(0, L, PB):
                        buf = ring.tile([128, PB], BF16, tag="obuf",
                                        name="obuf", bufs=4)
                        if ci % 5 == 4:
                            nc.gpsimd.tensor_scalar(
                                out=buf, in0=mid[:, q, b, ch:ch + PB],
                                scalar1=alpha[:, q, b:b + 1],
                                scalar2=beta[:, q, b:b + 1],
                                op0=OP.mult, op1=OP.add)
                        else:
                            nc.scalar.activation(
                                out=buf, in_=mid[:, q, b, ch:ch + PB],
                                func=AF.Identity,
                                bias=beta[:, q, b:b + 1],
                                scale=alpha[:, q, b:b + 1])
                        nc.vector.tensor_tensor(
                            out=buf, in0=buf,
                            in1=xpad[:, q, b, 1 + ch:1 + ch + PB],
                            op=OP.add)
                        nc.vector.tensor_scalar(
                            out=buf, in0=buf, scalar1=1.0, scalar2=-1.0,
                            op0=OP.min, op1=OP.max)
                        nc.sync.dma_start(
                            out=out_d[b, q * 128:(q + 1) * 128, ch:ch + PB],
                            in_=buf)
                        ci += 1

    nc.finalize()
    return nc


_NC_CACHE = {}


def _get_nc():
    if "full" not in _NC_CACHE:
        _NC_CACHE["full"] = build()
    return _NC_CACHE["full"]


def _prep_inputs(w1, g1, b1, w2, g2, b2, fc1, fc2):
    bf16 = ml_dtypes.bfloat16
    fp8 = ml_dtypes.float8_e4m3

    def wprep(w, dt):
        # [cout, cin, k] -> sign -> [ci, k, p, q, co]
        ws = np.sign(w).astype(np.float32).reshape(CB, 128, CB, 128, K)
        return np.ascontiguousarray(ws.transpose(3, 4, 2, 0, 1)).astype(dt)

    w1t = wprep(w1, bf16)
    w2t = wprep(w2, fp8)
    gb1 = np.ascontiguousarray(
        np.stack([g1.reshape(CB, 128), b1.reshape(CB, 128)], axis=-1).transpose(1, 0, 2)
    ).astype(np.float32)
    gb2 = np.ascontiguousarray(
        np.stack([g2.reshape(CB, 128), b2.reshape(CB, 128)], axis=-1).transpose(1, 0, 2)
    ).astype(np.float32)
    fc1t = np.ascontiguousarray(
        fc1.reshape(64, CB, 128).transpose(2, 1, 0)).astype(np.float32)
    fc2t = np.ascontiguousarray(
        fc2.reshape(CB, 128, 64).transpose(2, 0, 1)).astype(np.float32)
    return w1t, w2t, gb1, gb2, fc1t, fc2t


def kernel(x, w1, g1, b1, w2, g2, b2, fc1, fc2,
           _trace=False, _tracekw=None):
    x = np.ascontiguousarray(np.asarray(x, dtype=np.float32))
    w1t, w2t, gb1, gb2, fc1t, fc2t = _prep_inputs(
        np.asarray(w1), np.asarray(g1), np.asarray(b1), np.asarray(w2),
        np.asarray(g2), np.asarray(b2), np.asarray(fc1), np.asarray(fc2))

    nc = _get_nc()
    in_maps = []
    for c in range(NCORES):
        in_maps.append({
            "x": x[c * B:(c + 1) * B],
            "w1t": w1t, "w2t": w2t, "gb1": gb1, "gb2": gb2,
            "fc1t": fc1t, "fc2t": fc2t,
        })
    kw = dict(_tracekw or {})
    res = run_bass_kernel_spmd(nc, in_maps, core_ids=list(range(NCORES)),
                               trace=_trace, **kw)
    out = np.concatenate([res.results[c]["out"] for c in range(NCORES)], axis=0)
    if _trace:
        return out.astype(np.float32), res
    return out.astype(np.float32)
